# revision 1
# baseline (speedup 1.0000x reference)
"""MoE (8 experts, top-2 routing) kernel for Trainium2 — expert-parallel on 8 NeuronCores.

Strategy (per the expert-parallel sharding hint):
  * The small gate (x @ Wg + bg -> softmax -> top-2) is computed host-side
    ("replicate the small gate"); the host plays the role of the all-to-all
    dispatch: tokens are gathered per selected expert, padded to a common
    capacity C, and each NeuronCore runs one expert's MLP over its token group.
  * Per core e: y = relu(x_e @ W1[e] + b1[e]) @ W2[e], scaled by the combine
    weight per token.  b2 is folded in on the host (out += combine_w @ b2),
    which is exact regardless of b2's value.
  * The host scatter-adds the 8 weighted expert outputs back to token order.

Device kernel (per core), all matmuls in float32r (fp32 data at full PE rate):
  mm1: hT[m*128:(m+1)*128, :] = relu(W1_chunk^T @ x^T + b1)    (H on partitions)
  mm2: y[tok_tile, :]        += hT_chunk^T @ W2_chunk           (tokens on partitions)
  mm2 accumulates 8 H-chunks (one "group") in PSUM, then flushes to an SBUF
  accumulator; 4 groups cover H=4096.  Weights stream through SBUF exactly once.
"""

import numpy as np

P = 128
D = 1024
H = 4096
E = 8
TOPK = 2
DK = D // P       # 8  contraction chunks for mm1
M = H // P        # 32 H chunks
GROUP = 8         # H chunks accumulated per PSUM residency
NGROUP = M // GROUP


def _token_tiles(C):
    """Split C (multiple of 128, >=256) into chunks of 256..512 (fp32r needs
    moving dim >= 256 for full PE rate)."""
    n = C // P
    if n < 4:
        return [C]
    k, r = divmod(n, 4)
    if r == 0:
        parts = [4] * k
    elif r == 1:
        parts = [4] * (k - 1) + [3, 2]
    else:
        parts = [4] * k + [r]
    # smallest first: the first PSUM group then needs the least input data,
    # so the tensor engine starts as soon as possible after launch
    return [p * P for p in sorted(parts)]


def _build_program(C):
    import concourse.mybir as mybir
    import concourse.tile as tile
    from concourse import bacc

    f32 = mybir.dt.float32
    f32r = mybir.dt.float32r
    Relu = mybir.ActivationFunctionType.Relu
    T = C // P
    tts = _token_tiles(C)

    nc = bacc.Bacc(
        "TRN2",
        target_bir_lowering=False,
        debug=False,
        enable_asserts=True,
        num_devices=E,
    )
    xt_d = nc.dram_tensor("xt", [P, DK, C], f32r, kind="ExternalInput").ap()
    w1_d = nc.dram_tensor("w1", [P, DK, H], f32r, kind="ExternalInput").ap()
    w2_d = nc.dram_tensor("w2", [H, D], f32r, kind="ExternalInput").ap()
    b1_d = nc.dram_tensor("b1", [P, M], f32, kind="ExternalInput").ap()
    wc_d = nc.dram_tensor("wc", [P, T], f32, kind="ExternalInput").ap()
    y_d = nc.dram_tensor("y", [C, D], f32, kind="ExternalOutput").ap()

    with tile.TileContext(nc) as tc:
        with (
            tc.tile_pool(name="const", bufs=1) as const,
            tc.tile_pool(name="w1p0", bufs=4) as w1p0,
            tc.tile_pool(name="w1p", bufs=2) as w1p,
            tc.tile_pool(name="w2p", bufs=GROUP) as w2p,
            tc.tile_pool(name="htp", bufs=GROUP) as htp,
            tc.tile_pool(name="php", bufs=3, space="PSUM") as php,
            tc.tile_pool(name="pyp", bufs=2, space="PSUM") as pyp,
            tc.tile_pool(name="pwp", bufs=1, space="PSUM") as pwp,
        ):
            # Two independent HWDGE queues: weights + y on the SP queue,
            # x / biases on the ACT queue.  Within each queue, descriptors
            # drain in emission order, so emit in order of first use.
            def load_w1(pool, lo_m, n_m, tag):
                t = pool.tile([P, DK, n_m * P], f32r, tag=tag)
                nc.sync.dma_start(t[:], w1_d[:, :, lo_m * P:(lo_m + n_m) * P])
                return t

            def load_w2(m):
                t = w2p.tile([P, D], f32r, tag="w2t")
                nc.sync.dma_start(t[:], w2_d[m * P:(m + 1) * P, :])
                return t

            # Single SP HWDGE queue; emit strictly in order of first use:
            # W1(m0-1), biases, x tile 0, W1(m2-7), x tiles 1-2, W2 group 0.
            w1_g0 = [load_w1(w1p0, 0, 2, "w1h")]
            b1t = const.tile([P, M], f32)
            nc.sync.dma_start(b1t[:], b1_d[:])
            wct = const.tile([P, T], f32)
            nc.sync.dma_start(wct[:], wc_d[:])
            xt = const.tile([P, DK, C], f32r)
            for dk in range(DK):
                nc.sync.dma_start(xt[:, dk, 0:tts[0]], xt_d[:, dk, 0:tts[0]])
            off = tts[0]
            tsz1 = tts[1] if len(tts) > 1 else 0
            for half in range(1, 4):
                w1_g0.append(load_w1(w1p0, 2 * half, 2, "w1h"))
                if tsz1:
                    # interleave tile-1 x chunks between W1 half-slabs so both
                    # streams stay just ahead of the tensor engine
                    for dk in range(2 * half, 2 * half + 2):
                        nc.sync.dma_start(
                            xt[:, dk, off:off + tsz1], xt_d[:, dk, off:off + tsz1]
                        )
            if tsz1:
                for dk in (0, 1):
                    nc.sync.dma_start(
                        xt[:, dk, off:off + tsz1], xt_d[:, dk, off:off + tsz1]
                    )
                off += tsz1
            for tsz in tts[2:]:
                for dk in range(DK):
                    nc.sync.dma_start(
                        xt[:, dk, off:off + tsz], xt_d[:, dk, off:off + tsz]
                    )
                off += tsz
            w2_pre = {m: load_w2(m) for m in range(GROUP)}

            y_sb = const.tile([P, T, D], f32)

            # PE warm-up on a zeroed tile: keeps the activity monitor from
            # throttling the clock while the first operands stream in.
            warm = const.tile([P, P], f32)
            nc.any.memset(warm[:], 0.0)
            pw = pwp.tile([P, P], f32, tag="pw")
            for _ in range(12):
                nc.tensor.matmul(pw[:], warm[:], warm[:], start=True, stop=True)

            for g in range(NGROUP):
                if g == 0:
                    w1t = w1_g0
                    w1_col0 = [0, 2, 4, 6]
                    n_cols = 2
                else:
                    w1t = [
                        load_w1(w1p, g * GROUP, 4, "w1s"),
                        load_w1(w1p, g * GROUP + 4, 4, "w1s"),
                    ]
                    w1_col0 = [g * GROUP, g * GROUP + 4]
                    n_cols = 4
                w2s = [
                    w2_pre.pop(m) if m in w2_pre else load_w2(m)
                    for m in range(g * GROUP, (g + 1) * GROUP)
                ]
                hts = [
                    htp.tile([P, C], f32r, tag="ht", name=f"ht_{g}_{mi}")
                    for mi in range(GROUP)
                ]
                # mm1, token-tile-major so x/W1 chunks are needed in stream order
                off = 0
                for tsz in tts:
                    for mi in range(GROUP):
                        m = g * GROUP + mi
                        w1s = w1t[mi // n_cols]
                        c = m - w1_col0[mi // n_cols]
                        ph = php.tile([P, 512], f32, tag="ph")
                        for dk in range(DK):
                            nc.tensor.matmul(
                                ph[:, :tsz],
                                w1s[:, dk, c * P:(c + 1) * P],
                                xt[:, dk, off:off + tsz],
                                start=(dk == 0),
                                stop=(dk == DK - 1),
                            )
                        nc.scalar.activation(
                            hts[mi][:, off:off + tsz], ph[:, :tsz], Relu,
                            bias=b1t[:, m:m + 1],
                        )
                    off += tsz
                for t in range(T):
                    py = pyp.tile([P, D], f32, tag="py")
                    for mi in range(GROUP):
                        for h2 in range(2):
                            nc.tensor.matmul(
                                py[:, h2 * 512:(h2 + 1) * 512],
                                hts[mi][:, t * P:(t + 1) * P],
                                w2s[mi][:, h2 * 512:(h2 + 1) * 512],
                                start=(mi == 0),
                                stop=(mi == GROUP - 1),
                            )
                    if g == 0:
                        nc.vector.tensor_copy(y_sb[:, t, :], py[:])
                    else:
                        nc.vector.tensor_add(y_sb[:, t, :], y_sb[:, t, :], py[:])
                    if g == NGROUP - 1:
                        # final flush for this token subtile: apply the combine
                        # weight and store while later subtiles still compute
                        nc.vector.tensor_scalar_mul(
                            y_sb[:, t, :], y_sb[:, t, :], wct[:, t:t + 1]
                        )
                        nc.sync.dma_start(y_d[t * P:(t + 1) * P, :], y_sb[:, t, :])
    nc.compile()
    return nc


def _route(x, Wg, bg):
    """Host gate: softmax over experts + stable top-2 (mirrors jax.lax.top_k
    tie-breaking: lowest index first)."""
    logits = x @ Wg + bg
    mx = logits.max(axis=1, keepdims=True)
    ex = np.exp(logits - mx)
    gate = ex / ex.sum(axis=1, keepdims=True)
    top2 = np.argsort(-gate, axis=1, kind="stable")[:, :TOPK]
    return gate, top2


def _pack_core_inputs(x, gate, idx, W1e, b1e, W2e, C):
    cnt = len(idx)
    xe = np.zeros((C, D), np.float32)
    xe[:cnt] = x[idx]
    wc = np.zeros((C,), np.float32)
    wc[:cnt] = gate[idx]
    xt = np.ascontiguousarray(xe.T.reshape(DK, P, C).transpose(1, 0, 2))
    w1 = np.ascontiguousarray(W1e.reshape(DK, P, H).transpose(1, 0, 2))
    b1 = np.ascontiguousarray(b1e.reshape(M, P).T)
    wcs = np.ascontiguousarray(wc.reshape(C // P, P).T)
    return {
        "xt": xt,
        "w1": w1,
        "w2": np.ascontiguousarray(W2e),
        "b1": b1,
        "wc": wcs,
    }


def kernel(x, Wg, bg, W1, b1, W2, b2):
    from concourse.bass_utils import run_bass_kernel_spmd

    x = np.asarray(x, np.float32)
    Wg = np.asarray(Wg, np.float32)
    bg = np.asarray(bg, np.float32)
    W1 = np.asarray(W1, np.float32)
    b1 = np.asarray(b1, np.float32)
    W2 = np.asarray(W2, np.float32)
    b2 = np.asarray(b2, np.float32)
    Ttok = x.shape[0]

    gate, top2 = _route(x, Wg, bg)
    expert_idx = []
    for e in range(E):
        sel = np.nonzero((top2 == e).any(axis=1))[0]
        expert_idx.append(sel)
    max_cnt = max(len(s) for s in expert_idx)
    C = max(256, -(-max_cnt // P) * P)

    nc = _build_program(C)
    in_maps = [
        _pack_core_inputs(x, gate[:, e], expert_idx[e], W1[e], b1[e], W2[e], C)
        for e in range(E)
    ]
    results = run_bass_kernel_spmd(nc, in_maps, core_ids=list(range(E))).results

    out = np.zeros((Ttok, D), np.float32)
    for e in range(E):
        idx = expert_idx[e]
        out[idx] += results[e]["y"][: len(idx)]
    # b2 contribution, folded on the host (exact: y*(w) device + w*b2 here)
    mask = np.zeros((Ttok, E), np.float32)
    np.put_along_axis(mask, top2, 1.0, axis=1)
    out += (gate * mask) @ b2
    return out



# revision 2
# speedup vs baseline: 1.1946x; 1.1946x over previous
"""MoE (8 experts, top-2 routing) kernel for Trainium2 — hidden-dim-sharded
(tensor-parallel) across 8 NeuronCores, all matmuls in bf16.

Why hidden-shard instead of expert-parallel: with one expert per core the
slowest core pads its token group to the global max (1152 of a 1024 mean),
wasting ~12% of the PE. Sharding the H=4096 hidden dim instead gives every
core a 512-wide slice of ALL 8 experts' W1/W2, so all cores do the exact
same amount of work (the full 2T = 8192 routed (token, expert) pairs at
1/8 the hidden width each), with zero token padding: both matmuls keep
tokens on the moving dim, which can be any size.

Per core c (h-slice c*512..(c+1)*512), per expert e (cnt_e tokens, exact):
  mm1: hT[hc*128:(hc+1)*128, tok] = relu(W1_slice^T @ x^T + b1)  (h on
       partitions, 4 h-chunks, contraction D=1024 via 8 chained matmuls)
  mm2: yT[dt*128:(dt+1)*128, tok] += W2_chunk^T @ hT_chunk       (d on
       partitions, 8 d-tiles, contraction 512 via 4 chained matmuls)
The host computes the gate/top-2 (replicated small gate), groups tokens
expert-major, sums the 8 partial yT outputs, applies the combine weight and
b2, and scatter-adds back to token order. Exactness: out = w*(y_dev) +
w*b2, so folding b2 on the host is exact.

Token chunks of <=512 (PSUM bank width) are software-pipelined: mm2 of
chunk i is emitted after mm1 of chunk i+1, so the PE never waits on the
scalar-engine relu. bf16 keeps the PE at full rate at any moving size and
halves HBM traffic (~50 MB/core total, well under the compute shadow).
"""

import numpy as np
import ml_dtypes

P = 128
D = 1024
H = 4096
E = 8
TOPK = 2
DK = D // P        # 8 contraction chunks for mm1
HS = H // E        # 512 hidden units per core
HC = HS // P       # 4 h-chunks per core
DT = D // P        # 8 output d-tiles
CH = 512           # max token chunk (PSUM bank = 512 fp32)


def _chunk_items(cnts):
    """Split each expert's token count into chunks <=CH tokens.
    Returns [(e, global_off, tsz, first_of_e)], expert-major order.
    Expert 0 leads with a small chunk so the first matmul can start as soon
    as possible after launch; the very last chunk is kept small so the
    final PSUM->SBUF->DRAM drain exposes almost no tail."""
    items = []
    off = 0
    for e, cnt in enumerate(cnts):
        if cnt == 0:
            continue
        sizes = []
        rem = cnt
        if e == 0 and cnt > 320:
            sizes.append(256)
            rem -= 256
        n = -(-rem // CH)
        base, r = divmod(rem, n)
        sizes += [base + 1] * r + [base] * (n - r)
        for k, s in enumerate(sizes):
            items.append((e, off, s, k == 0))
            off += s
    e, o, s, f = items[-1]
    if s > 224 and not f:
        items[-1] = (e, o, s - 128, f)
        items.append((e, o + s - 128, 128, False))
    return items


def _build_program(cnts):
    import concourse.mybir as mybir
    import concourse.tile as tile
    from concourse import bacc

    f32 = mybir.dt.float32
    bf16 = mybir.dt.bfloat16
    Relu = mybir.ActivationFunctionType.Relu
    TOT = sum(cnts)
    items = _chunk_items(cnts)

    nc = bacc.Bacc(
        "TRN2",
        target_bir_lowering=False,
        debug=False,
        enable_asserts=True,
        num_devices=E,
    )
    xg_d = nc.dram_tensor("xg", [P, DK, TOT], bf16, kind="ExternalInput").ap()
    w1_d = nc.dram_tensor("w1", [P, DK, E * HS], bf16, kind="ExternalInput").ap()
    w2_d = nc.dram_tensor("w2", [P, E * HC, D], bf16, kind="ExternalInput").ap()
    b1_d = nc.dram_tensor("b1", [P, E * HC], f32, kind="ExternalInput").ap()
    y_d = nc.dram_tensor("y", [P, DT, TOT], bf16, kind="ExternalOutput").ap()

    with tile.TileContext(nc) as tc:
        with (
            tc.tile_pool(name="const", bufs=1) as const,
            tc.tile_pool(name="w1p", bufs=2) as w1p,
            tc.tile_pool(name="w2p", bufs=2) as w2p,
            tc.tile_pool(name="xgp", bufs=4) as xgp,
            tc.tile_pool(name="htp", bufs=3) as htp,
            tc.tile_pool(name="ysp", bufs=3) as ysp,
            tc.tile_pool(name="php", bufs=3, space="PSUM") as php,
            tc.tile_pool(name="pyp", bufs=4, space="PSUM") as pyp,
            tc.tile_pool(name="pwp", bufs=1, space="PSUM") as pwp,
        ):
            # Input stream on the SP HWDGE queue, y output stream on the
            # Pool-engine queue; each drains in emission order.
            def load_w1(e, split):
                t = w1p.tile([P, DK, HS], bf16, tag="w1")
                if split:
                    # first h-chunk lands first so chain hc=0 starts early
                    nc.sync.dma_start(
                        t[:, :, 0:P], w1_d[:, :, e * HS:e * HS + P]
                    )
                else:
                    nc.sync.dma_start(t[:], w1_d[:, :, e * HS:(e + 1) * HS])
                return t

            def load_w1_rest(e, t):
                nc.sync.dma_start(
                    t[:, :, P:HS], w1_d[:, :, e * HS + P:(e + 1) * HS]
                )

            def load_w2(e):
                t = w2p.tile([P, HC, D], bf16, tag="w2")
                nc.sync.dma_start(t[:], w2_d[:, e * HC:(e + 1) * HC, :])
                return t

            def load_xg(off, tsz):
                t = xgp.tile([P, DK, CH], bf16, tag="xg")
                nc.sync.dma_start(t[:, :, 0:tsz], xg_d[:, :, off:off + tsz])
                return t

            b1t = const.tile([P, E * HC], f32)
            w1_tiles = {}
            w2_tiles = {}

            # PE warm-up on a zeroed tile: ramps the p-state clock while the
            # first weight/token DMAs stream in.
            warm = const.tile([P, P], f32)
            nc.any.memset(warm[:], 0.0)
            pw = pwp.tile([P, P], f32, tag="pw")

            ht_tiles = {}

            def mm1(i):
                e, off, tsz, first = items[i]
                xgt = load_xg(off, tsz)
                if first and e == 0:
                    load_w1_rest(e, w1_tiles[e])
                    nc.sync.dma_start(b1t[:], b1_d[:])
                if first:
                    w2_tiles[e] = load_w2(e)
                w1t = w1_tiles[e]
                htt = htp.tile([P, HC, CH], bf16, tag="ht")
                ht_tiles[i] = htt
                for hc in range(HC):
                    ph = php.tile([P, CH], f32, tag="ph")
                    for dk in range(DK):
                        nc.tensor.matmul(
                            ph[:, 0:tsz],
                            w1t[:, dk, hc * P:(hc + 1) * P],
                            xgt[:, dk, 0:tsz],
                            start=(dk == 0),
                            stop=(dk == DK - 1),
                        )
                    nc.scalar.activation(
                        htt[:, hc, 0:tsz], ph[:, 0:tsz], Relu,
                        bias=b1t[:, e * HC + hc:e * HC + hc + 1],
                    )

            def mm2(i):
                e, off, tsz, first = items[i]
                htt = ht_tiles.pop(i)
                w2t = w2_tiles[e]
                yst = ysp.tile([P, DT, CH], bf16, tag="ys")
                for dt in range(DT):
                    py = pyp.tile([P, CH], f32, tag="py")
                    for hc in range(HC):
                        nc.tensor.matmul(
                            py[:, 0:tsz],
                            w2t[:, hc, dt * P:(dt + 1) * P],
                            htt[:, hc, 0:tsz],
                            start=(hc == 0),
                            stop=(hc == HC - 1),
                        )
                    if dt % 2 == 0:
                        nc.vector.tensor_copy(yst[:, dt, 0:tsz], py[:, 0:tsz])
                    else:
                        nc.scalar.copy(yst[:, dt, 0:tsz], py[:, 0:tsz])
                nc.gpsimd.dma_start(y_d[:, :, off:off + tsz], yst[:, :, 0:tsz])

            # first expert's w1 h-chunk 0 leads the input queue
            w1_tiles[items[0][0]] = load_w1(items[0][0], split=True)
            for _ in range(12):
                nc.tensor.matmul(pw[:], warm[:], warm[:], start=True, stop=True)

            for i in range(len(items)):
                e, off, tsz, first = items[i]
                if first and e != items[0][0]:
                    w1_tiles[e] = load_w1(e, split=False)
                mm1(i)
                if i > 0:
                    mm2(i - 1)
            mm2(len(items) - 1)
    nc.compile()
    return nc, items


def _route(x, Wg, bg):
    """Host gate: softmax over experts + stable top-2 (mirrors jax.lax.top_k
    tie-breaking: lowest index first)."""
    logits = x @ Wg + bg
    mx = logits.max(axis=1, keepdims=True)
    ex = np.exp(logits - mx)
    gate = ex / ex.sum(axis=1, keepdims=True)
    top2 = np.argsort(-gate, axis=1, kind="stable")[:, :TOPK]
    return gate, top2


def kernel(x, Wg, bg, W1, b1, W2, b2):
    from concourse.bass_utils import run_bass_kernel_spmd

    bf = ml_dtypes.bfloat16
    x = np.asarray(x, np.float32)
    Wg = np.asarray(Wg, np.float32)
    bg = np.asarray(bg, np.float32)
    W1 = np.asarray(W1, np.float32)
    b1 = np.asarray(b1, np.float32)
    W2 = np.asarray(W2, np.float32)
    b2 = np.asarray(b2, np.float32)
    Ttok = x.shape[0]

    gate, top2 = _route(x, Wg, bg)
    expert_idx = [np.nonzero((top2 == e).any(axis=1))[0] for e in range(E)]
    cnts = [len(s) for s in expert_idx]
    TOT = sum(cnts)
    order = np.concatenate([s for s in expert_idx if len(s)])
    offs = np.cumsum([0] + cnts)

    nc, _items = _build_program(cnts)

    # xg: x^T gathered expert-major, D-chunk tiled: xg[p, dk, j] =
    # x[order[j], dk*128 + p].  Identical for every core.
    xg = np.ascontiguousarray(
        x[order].astype(bf).T.reshape(DK, P, TOT).transpose(1, 0, 2)
    )
    W1b = W1.astype(bf)
    W2b = W2.astype(bf)
    in_maps = []
    for c in range(E):
        # w1[p, dk, e*512 + h] = W1[e, dk*128+p, c*512+h]
        w1c = np.ascontiguousarray(
            W1b[:, :, c * HS:(c + 1) * HS]
            .reshape(E, DK, P, HS).transpose(2, 1, 0, 3).reshape(P, DK, E * HS)
        )
        # w2[p, e*4+hc, d] = W2[e, c*512 + hc*128 + p, d]
        w2c = np.ascontiguousarray(
            W2b[:, c * HS:(c + 1) * HS, :]
            .reshape(E, HC, P, D).transpose(2, 0, 1, 3).reshape(P, E * HC, D)
        )
        # b1s[p, e*4+hc] = b1[e, c*512 + hc*128 + p]
        b1c = np.ascontiguousarray(
            b1[:, c * HS:(c + 1) * HS].reshape(E, HC, P).transpose(2, 0, 1)
            .reshape(P, E * HC).astype(np.float32)
        )
        in_maps.append({"xg": xg, "w1": w1c, "w2": w2c, "b1": b1c})

    results = run_bass_kernel_spmd(nc, in_maps, core_ids=list(range(E))).results

    # Sum the 8 partial yT, apply combine weights, scatter back to tokens.
    acc = np.zeros((P, DT, TOT), np.float32)
    for c in range(E):
        acc += results[c]["y"].astype(np.float32)
    yT = acc.transpose(1, 0, 2).reshape(D, TOT)   # yT[d, j]
    out = np.zeros((Ttok, D), np.float32)
    for e in range(E):
        idx = expert_idx[e]
        if len(idx) == 0:
            continue
        blk = yT[:, offs[e]:offs[e + 1]].T
        out[idx] += gate[idx, e:e + 1] * blk
    # b2 contribution, folded on the host (exact: w*y device + w*b2 here)
    mask = np.zeros((Ttok, E), np.float32)
    np.put_along_axis(mask, top2, 1.0, axis=1)
    out += (gate * mask) @ b2
    return out
